# revision 30
# baseline (speedup 1.0000x reference)
"""Trainium2 Bass kernel for nn_EnhancedSubtractionUnit.

B=8, C=256, H=W=64. Data-parallel over batch: 1 sample per NeuronCore (8 cores).

Per-core pipeline (channel-major layout [C_part, H, W], C split into 2 blocks
of 128 partitions; spatial padded to 66x66 for SAME 3x3 convs):

Front end is software-pipelined by 8-row strip (nt) so the offset/gather
chain hides entirely under conv1/conv2's PE stream:
    stage nt:   conv1(nt)   512->256 bf16 hi/lo 3-matmul scheme (BN folded);
                            f32r/bf16-only schemes are too coarse for the
                            offset path (verified: 2mm -> rel err 0.3)
    stage nt-1: conv2(nt-1) 256->2 offsets, hi/lo M-packed (2 matmuls/chunk),
                            grid scale (x32) folded into weights
    stage nt-2: grid(nt-2)  PE-transpose offsets to pixel-partition layout,
                            exact floor + validity + bilinear weights + gather
                            indices on DVE, issue indirect-DMA row gathers
    stage nt-3: diff(nt-3)  bilinear combine (bf16 gather table, f32 acc),
                            PE-transpose back to channel-major,
                            diff = x_low - aligned (stored bf16)
Back end (unchanged structure, bf16 storage throughout):
    3x DynamicScaleConv branches on diff (bf16 matmuls), SE pooling free via
    ACT accum_out, SE matvecs on PE, fused += xd * s; attention conv (bf16,
    weights replicated to M=128), sigmoid, out = attn*diff + x_low.
"""
import os
import sys

sys.path.insert(0, "/opt/trn_rl_repo")

import numpy as np
import concourse.bass as bass
import concourse.bacc as bacc
import concourse.tile as tile
from concourse import mybir
from concourse.bass_utils import run_bass_kernel_spmd

F32 = mybir.dt.float32
BF16 = mybir.dt.bfloat16
I32 = mybir.dt.int32
ALU = mybir.AluOpType
ACT = mybir.ActivationFunctionType

B, C, H, W = 8, 256, 64, 64
HW = H * W
PH, PW = H + 2, W + 2  # padded spatial
NCORES = 8
EPS = 1e-5
TBL_ROWS = 4160  # >= 4098 guard-padded gather table rows

_nc_cache = {}


def _load_consts(nc, tc, prm):
    """Allocate + DMA-load all load-once constants. Returns (pool, dict)."""
    pc = tc.alloc_tile_pool(name="const", bufs=1)
    cn = {}

    def cload(name, shape, dt, src):
        t = pc.tile(shape, dt, name=name)
        nc.sync.dma_start(t[:], src)
        cn[name] = t

    cload("w2p_sb", [128, 72], BF16, prm["w2p"][:])
    cload("bxy_sb", [128, 64], F32, prm["bxy"][:])
    cload("id_sb", [128, 128], F32, prm["ident"][:])
    cload("b1_sb", [128, 2], F32, prm["b1"][:])
    cload("saw_sb", [128, 2304], BF16, prm["sawT"][:])
    cload("sab_sb", [128, 1], F32, prm["sab_bc"][:])
    cload("db_sb", [128, 6], F32, prm["db2"][:])
    cload("se1_sb", [128, 384], F32, prm["se1T"][:])
    cload("se2_sb", [64, 768], F32, prm["se2T"][:])
    cload("se1b_sb", [64, 3], F32, prm["se1b2"][:])
    cload("se2b_sb", [128, 6], F32, prm["se2b2"][:])
    return pc, cn


def _emit_body(nc, tc, prm, cn, first_iter=True):
    """Emit one full forward pass. prm: dict of DRAM param handles.

    first_iter=False skips pad-border memsets: tile SBUF addresses are
    identical across For_i iterations, borders are only ever written by
    the memsets, and interiors are fully rewritten each pass.
    """
    ctx_pools = []

    def memset0(ap):
        if first_iter:
            nc.gpsimd.memset(ap, 0.0)

    def memset_border(t):
        # zero the whole padded tile on the first iteration (interiors are
        # rewritten every pass; only the pad ring survives). Column-strided
        # partial memsets proved unreliable on HW, so keep it simple -- this
        # only costs iteration-1 time, not the steady-state loop.
        if first_iter:
            nc.gpsimd.memset(t[:], 0.0)

    def pool(name, bufs=1, space="SBUF"):
        p = tc.alloc_tile_pool(name=name, bufs=bufs, space=space)
        ctx_pools.append(p)
        return p

    pc = pool("scratch", 1)
    ppsum = pool("ppsum", 2, space="PSUM")
    ptpsum = pool("ptpsum", 2, space="PSUM")
    psmall = pool("psmall", 1, space="PSUM")
    # diff_pad + gather work tiles outlive the conv-era transient pool
    # (the pipeline drain overlaps the first dw-conv groups) -> allocate below
    pdiff = pool("pdiff", 1)
    pgather = pool("pgather", 1)
    ptrans = tc.alloc_tile_pool(name="ptrans", bufs=1)  # dies after conv2(7)

    w2p_sb = cn["w2p_sb"]
    bxy_sb = cn["bxy_sb"]
    id_sb = cn["id_sb"]
    b1_sb = cn["b1_sb"]
    saw_sb = cn["saw_sb"]
    sab_sb = cn["sab_sb"]
    db_sb = cn["db_sb"]
    se1_sb = cn["se1_sb"]
    se2_sb = cn["se2_sb"]
    se1b_sb = cn["se1b_sb"]
    se2b_sb = cn["se2b_sb"]

    diff_pad = []
    for co in range(2):
        t = pdiff.tile([128, PH, PW], BF16, name=f"diff_pad{co}")
        memset_border(t)
        diff_pad.append(t)

    # ---------------- conv-era tiles ----------------
    # padded bf16 hi/lo inputs, split on host: order [xl0, xl1, xh0, xh1]
    xcat_hi = []
    xcat_lo = []
    for b4, (pname, cio) in enumerate(
            [("xl", 0), ("xl", 1), ("xh", 0), ("xh", 1)]):
        thi = ptrans.tile([128, PH, PW], BF16, name=f"xhi{b4}")
        memset_border(thi)
        xcat_hi.append((thi, f"{pname}hi", cio))
        tlo = ptrans.tile([128, PH, PW], BF16, name=f"xlo{b4}")
        memset_border(tlo)
        xcat_lo.append((tlo, f"{pname}lo", cio))
    # chunked loads round-robin over the 2 HW-DGE queues (SP, Act): chunk 0
    # of all 8 tensors first so conv1(0) starts early, then the conv1
    # weights, then chunks 1-2. (The gpsimd soft-DGE queue is left to the
    # indirect gathers: routing bulk loads there was both slow on HW and
    # implicated in a core-timing-dependent race.)
    queues = [nc.sync, nc.scalar]
    qi = [0]

    def qrr():
        q = queues[qi[0] % len(queues)]
        qi[0] += 1
        return q

    def load_chunk(r0, r1):
        for t, pname, cio in xcat_hi + xcat_lo:
            qrr().dma_start(t[:, 1 + r0:1 + r1, 1:65],
                            prm[pname][cio * 128:(cio + 1) * 128, r0:r1, :])

    load_chunk(0, 24)
    w1s = []
    for co in range(2):
        w1h_sb = ptrans.tile([128, 4608], BF16, name=f"w1h_sb{co}")
        qrr().dma_start(w1h_sb[:], prm["w1Th"][:, co * 4608:(co + 1) * 4608])
        w1l_sb = ptrans.tile([128, 4608], BF16, name=f"w1l_sb{co}")
        qrr().dma_start(w1l_sb[:], prm["w1Tl"][:, co * 4608:(co + 1) * 4608])
        w1s.append((w1h_sb, w1l_sb))
    load_chunk(24, 48)
    load_chunk(48, 64)
    xcat_hi = [t for t, _, _ in xcat_hi]
    xcat_lo = [t for t, _, _ in xcat_lo]

    h_hi = []
    h_lo = []
    for co in range(2):
        t = ptrans.tile([128, PH, PW], BF16, name=f"h_hi{co}")
        memset_border(t)
        h_hi.append(t)
        t = ptrans.tile([128, PH, PW], BF16, name=f"h_lo{co}")
        memset_border(t)
        h_lo.append(t)

    # ---------------- grid-math persistent scratch ----------------
    G = [128, 32]

    def f32t(name):
        return pc.tile(G, F32, name=name)

    ixiy = pc.tile([128, 64], F32, name="ixiy")
    xi_i = pc.tile(G, I32, name="xi_i")
    yi_i = pc.tile(G, I32, name="yi_i")
    fx0 = f32t("fx0")
    fy0 = f32t("fy0")
    corr = f32t("corr")
    corr2 = f32t("corr2")
    wx = f32t("wx")
    wy = f32t("wy")
    vx0 = f32t("vx0")
    vx1 = f32t("vx1")
    vy0 = f32t("vy0")
    vy1 = f32t("vy1")
    va = f32t("va")
    vb = f32t("vb")
    xc1 = f32t("xc1")
    yc0 = f32t("yc0")
    yc1 = f32t("yc1")
    idxA_f = f32t("idxA_f")
    idxB_f = f32t("idxB_f")
    idxA = pc.tile(G, I32, name="idxA")
    idxB = pc.tile(G, I32, name="idxB")
    u = f32t("u")
    v = f32t("v")
    wxv = f32t("wxv")
    wyv = f32t("wyv")
    w00 = f32t("w00")
    w01 = f32t("w01")
    w10 = f32t("w10")
    w11 = f32t("w11")

    # ---------------- pipeline stage emitters ----------------
    def conv1_strip(nt):
        for co in range(2):
            w1h_sb, w1l_sb = w1s[co]
            ps = ppsum.tile([128, 512], F32, name="c1psum", tag="c1psum")
            first = True
            for t9 in range(9):
                dy, dx = t9 // 3 - 1, t9 % 3 - 1
                for ci in range(4):
                    col = (t9 * 4 + ci) * 128
                    rhs_hi = xcat_hi[ci][:, nt * 8 + 1 + dy:nt * 8 + 9 + dy,
                                         1 + dx:65 + dx]
                    rhs_lo = xcat_lo[ci][:, nt * 8 + 1 + dy:nt * 8 + 9 + dy,
                                         1 + dx:65 + dx]
                    last = (t9 == 8 and ci == 3)
                    nc.tensor.matmul(ps[:], w1h_sb[:, col:col + 128], rhs_hi,
                                     start=first, stop=False)
                    nc.tensor.matmul(ps[:], w1h_sb[:, col:col + 128], rhs_lo,
                                     start=False, stop=False)
                    nc.tensor.matmul(ps[:], w1l_sb[:, col:col + 128], rhs_hi,
                                     start=False, stop=last)
                    first = False
            # h_hi = relu(ps + b1) via ACT (bf16 write rounds);
            # h_lo = max(ps + b1, 0) - h_hi via one DVE op reading PSUM
            hiv = h_hi[co][:, nt * 8 + 1:nt * 8 + 9, 1:65]
            nc.scalar.activation(hiv, ps[:], ACT.Relu,
                                 bias=b1_sb[:, co:co + 1], scale=1.0)
            hstg = ptrans.tile([128, 512], F32, name="hstg", tag="hstg", bufs=2)
            nc.scalar.activation(hstg[:], ps[:], ACT.Relu,
                                 bias=b1_sb[:, co:co + 1], scale=1.0)
            nc.vector.tensor_sub(h_lo[co][:, nt * 8 + 1:nt * 8 + 9, 1:65],
                                 hstg[:], hiv)

    def conv2_strip(m):
        # psA rows = [w2h*h_hi (2) ; w2l*h_hi (2)], psB = w2h*h_lo. The
        # pair-sum across partitions happens post-transpose, in the free dim.
        # both accumulators packed into one PSUM bank: psA at partitions 0-3
        # (col group 0), psB at 32-33 (col group 1) -> they run concurrently
        ps6 = psmall.tile([34, 512], F32, name="c2ps", tag="c2ps")
        psA = ps6[0:4, :]
        psB = ps6[32:34, :]
        for t9 in range(9):
            dy, dx = t9 // 3 - 1, t9 % 3 - 1
            for ci in range(2):
                c = t9 * 2 + ci
                rhs_hi = h_hi[ci][:, m * 8 + 1 + dy:m * 8 + 9 + dy, 1 + dx:65 + dx]
                rhs_lo = h_lo[ci][:, m * 8 + 1 + dy:m * 8 + 9 + dy, 1 + dx:65 + dx]
                first = (c == 0)
                last = (c == 17)
                nc.tensor.matmul(psA, w2p_sb[:, c * 4:c * 4 + 4],
                                 rhs_hi, start=first, stop=last)
                nc.tensor.matmul(psB, w2p_sb[:, c * 4:c * 4 + 2],
                                 rhs_lo, start=first, stop=last,
                                 tile_position=(0, 32))
        offA = pgather.tile([4, 512], F32, name="offA", tag="offA", bufs=2)
        offB = pgather.tile([2, 512], F32, name="offB", tag="offB", bufs=2)
        nc.vector.tensor_copy(offA[:], psA)
        nc.vector.tensor_copy(offB[:], psB)
        return offA, offB

    def grid_strip(m, offAB):
        offA, offB = offAB
        # pixel-partition layout: pixel p = j*128 + i -> [i, j], j in [0,32)
        pst6 = ptpsum.tile([128, 4, 6], F32, name="offT_psum", tag="offT",
                           bufs=2)
        for jj in range(4):
            nc.tensor.transpose(pst6[:, jj, 0:4],
                                offA[:, jj * 128:(jj + 1) * 128], id_sb[:4, :4])
            nc.tensor.transpose(pst6[:, jj, 4:6],
                                offB[:, jj * 128:(jj + 1) * 128], id_sb[:2, :2])
        J = slice(4 * m, 4 * m + 4)        # j-cols of [128,32] tiles
        X = slice(8 * m, 8 * m + 8)        # cols of ixiy [128,64]
        i3 = ixiy[:, X].rearrange("p (j c) -> p j c", c=2)
        # ix/iy = 32*offset + base (scale folded into w2 on host; bxy = base)
        nc.vector.tensor_copy(i3, pst6[:, :, 0:2])
        nc.vector.tensor_add(i3, i3, pst6[:, :, 2:4])
        nc.vector.tensor_add(i3, i3, pst6[:, :, 4:6])
        nc.vector.tensor_add(ixiy[:, X], ixiy[:, X], bxy_sb[:, X])
        ix = ixiy[:, 8 * m:8 * m + 7:2]
        iy = ixiy[:, 8 * m + 1:8 * m + 8:2]

        # exact floor via int cast + correction
        nc.vector.tensor_copy(xi_i[:, J], ix)
        nc.vector.tensor_copy(fx0[:, J], xi_i[:, J])
        nc.vector.tensor_tensor(corr[:, J], fx0[:, J], ix, op=ALU.is_gt)
        nc.vector.tensor_sub(fx0[:, J], fx0[:, J], corr[:, J])
        nc.vector.tensor_copy(yi_i[:, J], iy)
        nc.vector.tensor_copy(fy0[:, J], yi_i[:, J])
        nc.vector.tensor_tensor(corr2[:, J], fy0[:, J], iy, op=ALU.is_gt)
        nc.vector.tensor_sub(fy0[:, J], fy0[:, J], corr2[:, J])

        nc.vector.tensor_sub(wx[:, J], ix, fx0[:, J])
        nc.vector.tensor_sub(wy[:, J], iy, fy0[:, J])

        def valid01(src, v0, v1):
            nc.vector.tensor_scalar(va[:, J], src[:, J], 0.0, None, op0=ALU.is_ge)
            nc.vector.tensor_scalar(vb[:, J], src[:, J], 63.0, None, op0=ALU.is_le)
            nc.vector.tensor_mul(v0[:, J], va[:, J], vb[:, J])
            nc.vector.tensor_scalar(va[:, J], src[:, J], -1.0, None, op0=ALU.is_ge)
            nc.vector.tensor_scalar(vb[:, J], src[:, J], 62.0, None, op0=ALU.is_le)
            nc.vector.tensor_mul(v1[:, J], va[:, J], vb[:, J])

        valid01(fx0, vx0, vx1)
        valid01(fy0, vy0, vy1)

        # clamped addresses (+1 guard-row shift folded into xc1)
        nc.vector.tensor_scalar(xc1[:, J], fx0[:, J], -1.0, 64.0,
                                op0=ALU.max, op1=ALU.min)
        nc.vector.tensor_scalar_add(xc1[:, J], xc1[:, J], 1.0)
        nc.vector.tensor_scalar(yc0[:, J], fy0[:, J], 0.0, 63.0,
                                op0=ALU.max, op1=ALU.min)
        nc.vector.tensor_scalar(yc1[:, J], fy0[:, J], 1.0, 0.0,
                                op0=ALU.add, op1=ALU.max)
        nc.vector.tensor_scalar_min(yc1[:, J], yc1[:, J], 63.0)

        nc.vector.scalar_tensor_tensor(idxA_f[:, J], yc0[:, J], 64.0, xc1[:, J],
                                       op0=ALU.mult, op1=ALU.add)
        nc.vector.scalar_tensor_tensor(idxB_f[:, J], yc1[:, J], 64.0, xc1[:, J],
                                       op0=ALU.mult, op1=ALU.add)
        nc.vector.tensor_copy(idxA[:, J], idxA_f[:, J])
        nc.vector.tensor_copy(idxB[:, J], idxB_f[:, J])

        # bilinear weights, validity folded in
        nc.vector.tensor_scalar(u[:, J], wx[:, J], -1.0, 1.0,
                                op0=ALU.mult, op1=ALU.add)
        nc.vector.tensor_mul(u[:, J], u[:, J], vx0[:, J])
        nc.vector.tensor_scalar(v[:, J], wy[:, J], -1.0, 1.0,
                                op0=ALU.mult, op1=ALU.add)
        nc.vector.tensor_mul(v[:, J], v[:, J], vy0[:, J])
        nc.vector.tensor_mul(wxv[:, J], wx[:, J], vx1[:, J])
        nc.vector.tensor_mul(wyv[:, J], wy[:, J], vy1[:, J])
        nc.vector.tensor_mul(w00[:, J], u[:, J], v[:, J])
        nc.vector.tensor_mul(w01[:, J], wxv[:, J], v[:, J])
        nc.vector.tensor_mul(w10[:, J], u[:, J], wyv[:, J])
        nc.vector.tensor_mul(w11[:, J], wxv[:, J], wyv[:, J])

        # issue the row gathers for this strip's 4 j-chunks now; the
        # bilinear combine happens one stage later
        gs = []
        for jj in range(4):
            j = 4 * m + jj
            gA = pgather.tile([128, 512], BF16, name="gA", tag="gA", bufs=8)
            gB = pgather.tile([128, 512], BF16, name="gB", tag="gB", bufs=8)
            nc.gpsimd.indirect_dma_start(
                out=gA[:], out_offset=None, in_=prm["xT2"][:],
                in_offset=bass.IndirectOffsetOnAxis(ap=idxA[:, j:j + 1], axis=0))
            nc.gpsimd.indirect_dma_start(
                out=gB[:], out_offset=None, in_=prm["xT2"][:],
                in_offset=bass.IndirectOffsetOnAxis(ap=idxB[:, j:j + 1], axis=0))
            gs.append((gA, gB))
        return gs

    def diff_strip(m, gs):
        xlw = []
        for co in range(2):
            t = pgather.tile([128, 8, 64], F32, name=f"xlw{co}",
                             tag=f"xlw{co}", bufs=2)
            nc.sync.dma_start(t[:], prm["xl"][co * 128:(co + 1) * 128,
                                              m * 8:m * 8 + 8, :])
            xlw.append(t)
        for jj in range(4):
            j = 4 * m + jj
            gA, gB = gs[jj]
            acc = pgather.tile([128, 256], F32, name="acc", tag="acc", bufs=2)
            nc.vector.tensor_scalar_mul(acc[:], gA[:, 0:256], w00[:, j:j + 1])
            nc.vector.scalar_tensor_tensor(acc[:], gA[:, 256:512],
                                           w01[:, j:j + 1], acc[:],
                                           op0=ALU.mult, op1=ALU.add)
            nc.vector.scalar_tensor_tensor(acc[:], gB[:, 0:256],
                                           w10[:, j:j + 1], acc[:],
                                           op0=ALU.mult, op1=ALU.add)
            nc.vector.scalar_tensor_tensor(acc[:], gB[:, 256:512],
                                           w11[:, j:j + 1], acc[:],
                                           op0=ALU.mult, op1=ALU.add)
            # transpose [128px, 256ch] to channel-major, diff = x_low - aligned
            for co in range(2):
                pt = ptpsum.tile([128, 128], F32, name="alT_psum", tag="alT")
                nc.tensor.transpose(pt[:], acc[:, co * 128:(co + 1) * 128],
                                    id_sb[:])
                nc.vector.tensor_sub(
                    diff_pad[co][:, 2 * j + 1:2 * j + 3, 1:65],
                    xlw[co][:, 2 * jj:2 * jj + 2, :], pt[:])

    # ---------------- dw-branch emitters ----------------
    def dw_alloc(k):
        dwk_sb = pback.tile([128, 4608], BF16, name="dwk_sb", tag="dwk",
                            bufs=2)
        nc.scalar.dma_start(dwk_sb[:], prm[f"dwT{k}"][:])
        xd = []
        pooled_parts = []
        for co in range(2):
            xd_t = pback.tile([128, HW], BF16, name=f"xd{co}", tag=f"xd{co}",
                              bufs=2)
            xd.append(xd_t)
            pp_t = pc.tile([128, 8], F32, name=f"pooled_parts{co}",
                           tag=f"pooled_parts{co}", bufs=2)
            pooled_parts.append(pp_t)
        return k, dwk_sb, xd, pooled_parts

    def dw_group(kctx, co, nt):
        k, dwk_sb, xd, pooled_parts = kctx
        ps = ppsum.tile([128, 512], F32, name="dwpsum", tag="c1psum")
        first = True
        for t9 in range(9):
            dy, dx = t9 // 3 - 1, t9 % 3 - 1
            for ci in range(2):
                col = ((t9 * 2 + ci) * 2 + co) * 128
                nc.tensor.matmul(
                    ps[:],
                    dwk_sb[:, col:col + 128],
                    diff_pad[ci][:, nt * 8 + 1 + dy:nt * 8 + 9 + dy,
                                 1 + dx:65 + dx],
                    start=first, stop=(t9 == 8 and ci == 1))
                first = False
        nc.scalar.activation(
            xd[co][:, nt * 512:(nt + 1) * 512], ps[:],
            ACT.Identity, bias=db_sb[:, 2 * k + co:2 * k + co + 1],
            scale=1.0, accum_out=pooled_parts[co][:, nt:nt + 1])

    # ---------------- front-end pipeline ----------------
    conv2_out = {}
    grid_out = {}
    for nt in range(8):
        conv1_strip(nt)
        if nt >= 1:
            conv2_out[nt - 1] = conv2_strip(nt - 1)
        if nt >= 2:
            grid_out[nt - 2] = grid_strip(nt - 2, conv2_out.pop(nt - 2))
        if nt >= 3:
            diff_strip(nt - 3, grid_out.pop(nt - 3))
    conv2_out[7] = conv2_strip(7)  # last user of h / ptrans

    ptrans.release()
    pback = tc.alloc_tile_pool(name="pback", bufs=1)
    ctx_pools.append(pback)
    fused_pad = []
    for co in range(2):
        t = pback.tile([128, PH, PW], BF16, name=f"fused_pad{co}")
        memset_border(t)
        fused_pad.append(t)

    # pipeline drain interleaved with the first dw-conv groups so the PE
    # FIFO never stalls on the gather chain's transposes
    k0 = dw_alloc(0)
    grid_out[6] = grid_strip(6, conv2_out.pop(6))
    dw_group(k0, 0, 0)
    dw_group(k0, 0, 1)
    diff_strip(5, grid_out.pop(5))
    dw_group(k0, 0, 2)
    grid_out[7] = grid_strip(7, conv2_out.pop(7))
    dw_group(k0, 0, 3)
    diff_strip(6, grid_out.pop(6))
    dw_group(k0, 1, 0)
    dw_group(k0, 1, 1)
    diff_strip(7, grid_out.pop(7))
    for nt in range(4, 8):
        dw_group(k0, 0, nt)
    for nt in range(2, 8):
        dw_group(k0, 1, nt)

    # Software-pipelined: emit branch k+1's matmuls before branch k's SE
    # chain so the tiny SE matvecs (which wait on pooled stats) never stall
    # the PE queue between the big conv groups. xd / pooled tags use bufs=2
    # so branch k's data survives while branch k+1 computes.
    def emit_dw_matmuls(k):
        kctx = dw_alloc(k)
        for co in range(2):
            for nt in range(8):
                dw_group(kctx, co, nt)
        return kctx[2], kctx[3]

    def emit_se(k, xd, pooled_parts):
        # SE block (tiny matvecs); mean 1/HW folded into se1T on host.
        # The 8->1 reduction rides the ACT accumulator (the DVE queue is
        # busy with fused updates right when this needs to run).
        pooled = []
        for co in range(2):
            p_t = pc.tile([128, 1], F32, name=f"pooled{co}", tag=f"pooled{co}")
            pdmp = pc.tile([128, 8], F32, name=f"pdump{co}", tag=f"pdump{co}",
                           bufs=2)
            nc.scalar.activation(pdmp[:], pooled_parts[co][:], ACT.Identity,
                                 scale=1.0, accum_out=p_t[:])
            pooled.append(p_t)
        pse = psmall.tile([128, 2], F32, name="pse", tag="pse")
        nc.tensor.matmul(pse[0:64, 0:1], se1_sb[:, k * 128:k * 128 + 64],
                         pooled[0][:], start=True, stop=False)
        nc.tensor.matmul(pse[0:64, 0:1], se1_sb[:, k * 128 + 64:k * 128 + 128],
                         pooled[1][:], start=False, stop=True)
        h1 = pc.tile([64, 1], F32, name="h1", tag="h1")
        nc.scalar.activation(h1[:], pse[0:64, 0:1], ACT.Relu,
                             bias=se1b_sb[:, k:k + 1], scale=1.0)
        for co in range(2):
            nc.tensor.matmul(pse[:, 1:2],
                             se2_sb[:, (k * 2 + co) * 128:(k * 2 + co + 1) * 128],
                             h1[:], start=True, stop=True)
            s_t = pc.tile([128, 1], F32, name=f"s{co}", tag=f"s{co}")
            nc.scalar.activation(s_t[:], pse[:, 1:2], ACT.Sigmoid,
                                 bias=se2b_sb[:, 2 * k + co:2 * k + co + 1],
                                 scale=1.0)
            # fused += xd * s (split in half-image chunks so the attention
            # conv's first strips unblock after the first chunk)
            xd3 = xd[co][:].rearrange("p (h w) -> p h w", h=H)
            for r0, r1 in [(0, 32), (32, 64)]:
                fslice = fused_pad[co][:, 1 + r0:1 + r1, 1:1 + W]
                if k == 0:
                    nc.vector.tensor_scalar_mul(
                        fslice, xd3[:, r0:r1, :], s_t[:])
                else:
                    nc.vector.scalar_tensor_tensor(
                        fslice, xd3[:, r0:r1, :], s_t[:], fslice,
                        op0=ALU.mult, op1=ALU.add)

    prev = (k0[2], k0[3])
    for k in range(1, 3):
        cur = emit_dw_matmuls(k)
        emit_se(k - 1, *prev)
        prev = cur
    emit_se(2, *prev)

    # ---------------- attention + final ----------------
    for nt in range(8):
        attn = pback.tile([128, 512], BF16, name="attn", tag="attn", bufs=2)
        ps = ppsum.tile([128, 512], F32, name="sapsum", tag="c1psum")
        first = True
        for t9 in range(9):
            dy, dx = t9 // 3 - 1, t9 % 3 - 1
            for ci in range(2):
                col = (t9 * 2 + ci) * 128
                nc.tensor.matmul(
                    ps[:],
                    saw_sb[:, col:col + 128],
                    fused_pad[ci][:, nt * 8 + 1 + dy:nt * 8 + 9 + dy,
                                  1 + dx:65 + dx],
                    start=first, stop=(t9 == 8 and ci == 1))
                first = False
        nc.scalar.activation(attn[:], ps[:], ACT.Sigmoid, bias=sab_sb[:, 0:1],
                             scale=1.0)
        for co in range(2):
            xlt = pback.tile([128, 512], F32, name="xlt", tag="xlt", bufs=2)
            nc.sync.dma_start(
                xlt[:], prm["xl"][co * 128:(co + 1) * 128, nt * 8:(nt + 1) * 8, :])
            ot = pback.tile([128, 512], F32, name="ot", tag="ot", bufs=2)
            nc.vector.tensor_mul(
                ot[:], attn[:],
                diff_pad[co][:, nt * 8 + 1:nt * 8 + 9, 1:65])
            nc.vector.tensor_add(ot[:], ot[:], xlt[:])
            nc.sync.dma_start(
                prm["out"][co * 128:(co + 1) * 128, nt * 512:(nt + 1) * 512],
                ot[:])

    for p in reversed(ctx_pools):
        p.release()


def _build(repeat):
    nc = bacc.Bacc()
    prm = {}

    def din(name, shape, dt=F32):
        prm[name] = nc.declare_dram_parameter(name, list(shape), dt,
                                              isOutput=False)

    din("xl", [C, H, W])
    for nm in ["xlhi", "xllo", "xhhi", "xhlo"]:
        din(nm, [C, H, W], BF16)
    din("xT2", [TBL_ROWS, 512], BF16)
    din("w1Th", [128, 9216], BF16)
    din("w1Tl", [128, 9216], BF16)
    din("b1", [128, 2])
    din("w2p", [128, 72], BF16)
    din("bxy", [128, 64])
    din("ident", [128, 128])
    for k in range(3):
        din(f"dwT{k}", [128, 4608], BF16)
    din("db2", [128, 6])
    din("se1T", [128, 384])
    din("se1b2", [64, 3])
    din("se2T", [64, 768])
    din("se2b2", [128, 6])
    din("sawT", [128, 2304], BF16)
    din("sab_bc", [128, 1])
    prm["out"] = nc.declare_dram_parameter("out", [C, HW], F32, isOutput=True)

    with tile.TileContext(nc) as tc:
        pc_const, cn = _load_consts(nc, tc, prm)
        _emit_body(nc, tc, prm, cn, first_iter=True)
        if repeat > 1:
            with tc.For_i(0, repeat - 1, 1):
                _emit_body(nc, tc, prm, cn, first_iter=False)
        pc_const.release()
    nc.finalize()
    return nc


def _prep_inputs(x_low, x_high, a1w, a1b, bn_g, bn_b, bn_m, bn_v, a2w, a2b,
                 dw, db, se1w, se1b, se2w, se2b, saw, sab):
    """Host-side weight prep shared by all cores + per-core activation prep."""
    import ml_dtypes
    f32 = np.float32
    bf16 = ml_dtypes.bfloat16
    # conv1 with BN folded
    scale = (bn_g / np.sqrt(bn_v + EPS)).astype(f32)  # [256]
    w1f = (a1w * scale[:, None, None, None]).astype(f32)  # [256,512,3,3]
    b1f = ((a1b - bn_m) * scale + bn_b).astype(f32)  # [256]
    # host lhsT layout [k(128), ty,tx, ci(4), co(2), m(128)] -> [128, 9216]
    arr = w1f.reshape(2, 128, 4, 128, 3, 3)  # [co, m, ci, k, ty, tx]
    w1T = np.ascontiguousarray(arr.transpose(3, 0, 4, 5, 2, 1)).reshape(128, 9216)
    w1Th = w1T.astype(bf16)
    w1Tl = (w1T - w1Th.astype(np.float32)).astype(bf16)
    b1h = np.ascontiguousarray(b1f.reshape(2, 128).T)  # [128, 2]

    # conv2, grid scale W/2 = 32 folded in; hi/lo M-packed [w2h|w2l] per chunk
    w2f = (a2w * 32.0).astype(f32)  # [2, 256, 3, 3]
    arr = w2f.reshape(2, 2, 128, 3, 3)  # [m, ci, k, ty, tx]
    w2T = np.ascontiguousarray(arr.transpose(2, 3, 4, 1, 0))  # [k,ty,tx,ci,m]
    w2h = w2T.astype(bf16)
    w2l = (w2T - w2h.astype(np.float32)).astype(bf16)
    w2p = np.concatenate([w2h.reshape(128, 18, 2), w2l.reshape(128, 18, 2)],
                         axis=2)  # [k, chunk, 4]
    w2p = np.ascontiguousarray(w2p).reshape(128, 72)

    # base grid (+a2b*32): pixel p = j*128+i ; h=p//64, w=p%64
    lin = np.linspace(-1.0, 1.0, 64, dtype=f32)
    pidx = (np.arange(32)[None, :] * 128 + np.arange(128)[:, None])  # [128,32]
    bx = ((lin[pidx // 64] + 1.0) * 32.0 - 0.5 + 32.0 * f32(a2b[0])).astype(f32)
    by = ((lin[pidx % 64] + 1.0) * 32.0 - 0.5 + 32.0 * f32(a2b[1])).astype(f32)
    bxy = np.empty((128, 64), f32)
    bxy[:, 0::2] = bx
    bxy[:, 1::2] = by

    # diff convs (bf16)
    dwT = []
    for k in range(3):
        arr = dw[k].astype(f32).reshape(2, 128, 2, 128, 3, 3)  # [co,m,ci,kk,ty,tx]
        dwT.append(np.ascontiguousarray(
            arr.transpose(3, 4, 5, 2, 0, 1)).reshape(128, 4608).astype(bf16))
    db2 = np.ascontiguousarray(db.astype(f32).reshape(3, 2, 128).transpose(2, 0, 1)
                               ).reshape(128, 6)

    # SE (mean 1/HW folded into se1T)
    se1T = np.ascontiguousarray(
        (se1w.astype(f32) / HW).transpose(0, 2, 1).reshape(3, 2, 128, 64)
        .transpose(2, 0, 1, 3)).reshape(128, 384)
    se1b2 = np.ascontiguousarray(se1b.astype(f32).T)  # [64, 3]
    se2T = np.ascontiguousarray(
        se2w.astype(f32).transpose(0, 2, 1).reshape(3, 64, 2, 128)
        .transpose(1, 0, 2, 3)).reshape(64, 768)
    se2b2 = np.ascontiguousarray(se2b.astype(f32).reshape(3, 2, 128)
                                 .transpose(2, 0, 1)).reshape(128, 6)

    # attention conv, weights replicated to M=128 (bf16)
    arr = saw.astype(f32).reshape(1, 2, 128, 3, 3)  # [m=1, ci, k, ty, tx]
    arr = np.broadcast_to(arr, (128, 2, 128, 3, 3))  # replicate m
    sawT = np.ascontiguousarray(
        arr.transpose(2, 3, 4, 1, 0)).reshape(128, 2304).astype(bf16)
    sab_bc = np.full((128, 1), f32(sab[0]), f32)

    shared = dict(w1Th=w1Th, w1Tl=w1Tl, b1=b1h, w2p=w2p, bxy=bxy,
                  ident=np.eye(128, dtype=f32),
                  dwT0=dwT[0], dwT1=dwT[1], dwT2=dwT[2], db2=db2,
                  se1T=se1T, se1b2=se1b2, se2T=se2T, se2b2=se2b2,
                  sawT=sawT, sab_bc=sab_bc)

    in_maps = []
    for b in range(B):
        xlb = np.ascontiguousarray(x_low[b].astype(f32))
        xhb = x_high[b].astype(f32)
        xlhi = xlb.astype(bf16)
        xllo = (xlb - xlhi.astype(f32)).astype(bf16)
        xhhi = xhb.astype(bf16)
        xhlo = (xhb - xhhi.astype(f32)).astype(bf16)
        XT = np.ascontiguousarray(xhb.reshape(C, HW).T)  # [4096, 256]
        XT2 = np.zeros((TBL_ROWS, 512), f32)
        XT2[1:1 + HW, :256] = XT
        XT2[0:HW, 256:] = XT
        m = dict(shared)
        m["xl"] = xlb
        m["xlhi"] = np.ascontiguousarray(xlhi)
        m["xllo"] = np.ascontiguousarray(xllo)
        m["xhhi"] = np.ascontiguousarray(xhhi)
        m["xhlo"] = np.ascontiguousarray(xhlo)
        m["xT2"] = XT2.astype(bf16)
        in_maps.append(m)
    return in_maps


_last_results = None


def kernel(**inputs):
    global _last_results
    repeat = int(os.environ.get("KERNEL_REPEAT", "1"))
    if repeat not in _nc_cache:
        _nc_cache[repeat] = _build(repeat)
    nc = _nc_cache[repeat]
    in_maps = _prep_inputs(**inputs)
    res = run_bass_kernel_spmd(nc, in_maps, list(range(NCORES)))
    _last_results = res
    out = np.stack([res.results[b]["out"].reshape(C, H, W) for b in range(B)])
    return out.astype(np.float32)


if __name__ == "__main__":
    import reference
    inputs = {k: np.asarray(v) for k, v in reference.setup_inputs().items()}
    expected = np.asarray(reference.reference(**inputs))
    actual = kernel(**inputs)
    err = np.abs(actual - expected).max()
    rel = err / np.abs(expected).max()
    print(f"abs err: {err:.4e}  rel err: {rel:.4e}")


# revision 32
# speedup vs baseline: 1.0937x; 1.0937x over previous
"""Trainium2 Bass kernel for nn_EnhancedSubtractionUnit.

B=8, C=256, H=W=64. Data-parallel over batch: 1 sample per NeuronCore (8 cores).

Per-core pipeline (channel-major layout [C_part, H, W], C split into 2 blocks
of 128 partitions; spatial padded to 66x66 for SAME 3x3 convs):

Front end is software-pipelined by 8-row strip (nt) so the offset/gather
chain hides entirely under conv1/conv2's PE stream:
    stage nt:   conv1(nt)   512->256 bf16 hi/lo 3-matmul scheme (BN folded);
                            f32r/bf16-only schemes are too coarse for the
                            offset path (verified: 2mm -> rel err 0.3)
    stage nt-1: conv2(nt-1) 256->2 offsets, hi/lo M-packed (2 matmuls/chunk),
                            grid scale (x32) folded into weights
    stage nt-2: grid(nt-2)  PE-transpose offsets to pixel-partition layout,
                            exact floor + validity + bilinear weights + gather
                            indices on DVE, issue indirect-DMA row gathers
    stage nt-3: diff(nt-3)  bilinear combine (bf16 gather table, f32 acc),
                            PE-transpose back to channel-major,
                            diff = x_low - aligned (stored bf16)
Back end (unchanged structure, bf16 storage throughout):
    3x DynamicScaleConv branches on diff (bf16 matmuls), SE pooling free via
    ACT accum_out, SE matvecs on PE, fused += xd * s; attention conv (bf16,
    weights replicated to M=128), sigmoid, out = attn*diff + x_low.
"""
import os
import sys

sys.path.insert(0, "/opt/trn_rl_repo")

import numpy as np
import concourse.bass as bass
import concourse.bacc as bacc
import concourse.tile as tile
from concourse import mybir
from concourse.bass_utils import run_bass_kernel_spmd

F32 = mybir.dt.float32
BF16 = mybir.dt.bfloat16
I32 = mybir.dt.int32
ALU = mybir.AluOpType
ACT = mybir.ActivationFunctionType

B, C, H, W = 8, 256, 64, 64
HW = H * W
PH, PW = H + 2, W + 2  # padded spatial
NCORES = 8
EPS = 1e-5
TBL_ROWS = 4160  # >= 4098 guard-padded gather table rows

_nc_cache = {}


def _load_consts(nc, tc, prm):
    """Allocate + DMA-load all load-once constants. Returns (pool, dict)."""
    pc = tc.alloc_tile_pool(name="const", bufs=1)
    cn = {}

    def cload(name, shape, dt, src):
        t = pc.tile(shape, dt, name=name)
        nc.sync.dma_start(t[:], src)
        cn[name] = t

    cload("w2p_sb", [128, 72], BF16, prm["w2p"][:])
    cload("bxy_sb", [128, 64], F32, prm["bxy"][:])
    cload("id_sb", [128, 128], F32, prm["ident"][:])
    cload("b1_sb", [128, 2], F32, prm["b1"][:])
    cload("saw_sb", [128, 2304], BF16, prm["sawT"][:])
    cload("sab_sb", [128, 1], F32, prm["sab_bc"][:])
    cload("db_sb", [128, 6], F32, prm["db2"][:])
    cload("se1_sb", [128, 384], F32, prm["se1T"][:])
    cload("se2_sb", [64, 768], F32, prm["se2T"][:])
    cload("se1b_sb", [64, 3], F32, prm["se1b2"][:])
    cload("se2b_sb", [128, 6], F32, prm["se2b2"][:])
    return pc, cn


def _emit_body(nc, tc, prm, cn, first_iter=True):
    """Emit one full forward pass. prm: dict of DRAM param handles.

    first_iter=False skips pad-border memsets: tile SBUF addresses are
    identical across For_i iterations, borders are only ever written by
    the memsets, and interiors are fully rewritten each pass.
    """
    ctx_pools = []

    def memset0(ap):
        if first_iter:
            nc.gpsimd.memset(ap, 0.0)

    def memset_border(t):
        # zero the whole padded tile on the first iteration (interiors are
        # rewritten every pass; only the pad ring survives). Column-strided
        # partial memsets proved unreliable on HW, so keep it simple -- this
        # only costs iteration-1 time, not the steady-state loop.
        if first_iter:
            nc.gpsimd.memset(t[:], 0.0)

    def pool(name, bufs=1, space="SBUF"):
        p = tc.alloc_tile_pool(name=name, bufs=bufs, space=space)
        ctx_pools.append(p)
        return p

    pc = pool("scratch", 1)
    ppsum = pool("ppsum", 2, space="PSUM")
    ptpsum = pool("ptpsum", 2, space="PSUM")
    psmall = pool("psmall", 1, space="PSUM")
    # diff_pad + gather work tiles outlive the conv-era transient pool
    # (the pipeline drain overlaps the first dw-conv groups) -> allocate below
    pdiff = pool("pdiff", 1)
    pgather = pool("pgather", 1)
    ptrans = tc.alloc_tile_pool(name="ptrans", bufs=1)  # dies after conv2(7)

    w2p_sb = cn["w2p_sb"]
    bxy_sb = cn["bxy_sb"]
    id_sb = cn["id_sb"]
    b1_sb = cn["b1_sb"]
    saw_sb = cn["saw_sb"]
    sab_sb = cn["sab_sb"]
    db_sb = cn["db_sb"]
    se1_sb = cn["se1_sb"]
    se2_sb = cn["se2_sb"]
    se1b_sb = cn["se1b_sb"]
    se2b_sb = cn["se2b_sb"]

    diff_pad = []
    for co in range(2):
        t = pdiff.tile([128, PH, PW], BF16, name=f"diff_pad{co}")
        memset_border(t)
        diff_pad.append(t)

    # ---------------- conv-era tiles ----------------
    # padded bf16 hi/lo inputs, split on host: order [xl0, xl1, xh0, xh1]
    xcat_hi = []
    xcat_lo = []
    for b4, (pname, cio) in enumerate(
            [("xl", 0), ("xl", 1), ("xh", 0), ("xh", 1)]):
        thi = ptrans.tile([128, PH, PW], BF16, name=f"xhi{b4}")
        memset_border(thi)
        xcat_hi.append((thi, f"{pname}hi", cio))
        tlo = ptrans.tile([128, PH, PW], BF16, name=f"xlo{b4}")
        memset_border(tlo)
        xcat_lo.append((tlo, f"{pname}lo", cio))
    # chunked loads round-robin over the 2 HW-DGE queues (SP, Act): chunk 0
    # of all 8 tensors first so conv1(0) starts early, then the conv1
    # weights, then chunks 1-2. (The gpsimd soft-DGE queue is left to the
    # indirect gathers: routing bulk loads there was both slow on HW and
    # implicated in a core-timing-dependent race.)
    queues = [nc.sync, nc.scalar]
    qi = [0]

    def qrr():
        q = queues[qi[0] % len(queues)]
        qi[0] += 1
        return q

    def load_chunk(r0, r1):
        for t, pname, cio in xcat_hi + xcat_lo:
            qrr().dma_start(t[:, 1 + r0:1 + r1, 1:65],
                            prm[pname][cio * 128:(cio + 1) * 128, r0:r1, :])

    load_chunk(0, 24)
    w1s = []
    for co in range(2):
        w1h_sb = ptrans.tile([128, 4608], BF16, name=f"w1h_sb{co}")
        qrr().dma_start(w1h_sb[:], prm["w1Th"][:, co * 4608:(co + 1) * 4608])
        w1l_sb = ptrans.tile([128, 4608], BF16, name=f"w1l_sb{co}")
        qrr().dma_start(w1l_sb[:], prm["w1Tl"][:, co * 4608:(co + 1) * 4608])
        w1s.append((w1h_sb, w1l_sb))
    load_chunk(24, 48)
    load_chunk(48, 64)
    xcat_hi = [t for t, _, _ in xcat_hi]
    xcat_lo = [t for t, _, _ in xcat_lo]

    h_hi = []
    h_lo = []
    for co in range(2):
        t = ptrans.tile([128, PH, PW], BF16, name=f"h_hi{co}")
        memset_border(t)
        h_hi.append(t)
        t = ptrans.tile([128, PH, PW], BF16, name=f"h_lo{co}")
        memset_border(t)
        h_lo.append(t)

    # ---------------- grid-math persistent scratch ----------------
    G = [128, 32]

    def f32t(name):
        return pc.tile(G, F32, name=name)

    ixiy = pc.tile([128, 64], F32, name="ixiy")
    xi_i = pc.tile(G, I32, name="xi_i")
    yi_i = pc.tile(G, I32, name="yi_i")
    fx0 = f32t("fx0")
    fy0 = f32t("fy0")
    corr = f32t("corr")
    corr2 = f32t("corr2")
    wx = f32t("wx")
    wy = f32t("wy")
    vx0 = f32t("vx0")
    vx1 = f32t("vx1")
    vy0 = f32t("vy0")
    vy1 = f32t("vy1")
    va = f32t("va")
    vb = f32t("vb")
    xc1 = f32t("xc1")
    yc0 = f32t("yc0")
    yc1 = f32t("yc1")
    idxA_f = f32t("idxA_f")
    idxB_f = f32t("idxB_f")
    idxA = pc.tile(G, I32, name="idxA")
    idxB = pc.tile(G, I32, name="idxB")
    u = f32t("u")
    v = f32t("v")
    wxv = f32t("wxv")
    wyv = f32t("wyv")
    w00 = f32t("w00")
    w01 = f32t("w01")
    w10 = f32t("w10")
    w11 = f32t("w11")

    # ---------------- pipeline stage emitters ----------------
    def conv1_strip(nt):
        for co in range(2):
            w1h_sb, w1l_sb = w1s[co]
            ps = ppsum.tile([128, 512], F32, name="c1psum", tag="c1psum")
            first = True
            for t9 in range(9):
                dy, dx = t9 // 3 - 1, t9 % 3 - 1
                for ci in range(4):
                    col = (t9 * 4 + ci) * 128
                    rhs_hi = xcat_hi[ci][:, nt * 8 + 1 + dy:nt * 8 + 9 + dy,
                                         1 + dx:65 + dx]
                    rhs_lo = xcat_lo[ci][:, nt * 8 + 1 + dy:nt * 8 + 9 + dy,
                                         1 + dx:65 + dx]
                    last = (t9 == 8 and ci == 3)
                    nc.tensor.matmul(ps[:], w1h_sb[:, col:col + 128], rhs_hi,
                                     start=first, stop=False)
                    nc.tensor.matmul(ps[:], w1h_sb[:, col:col + 128], rhs_lo,
                                     start=False, stop=False)
                    nc.tensor.matmul(ps[:], w1l_sb[:, col:col + 128], rhs_hi,
                                     start=False, stop=last)
                    first = False
            # h_hi = relu(ps + b1) via ACT (bf16 write rounds);
            # h_lo = max(ps + b1, 0) - h_hi via one DVE op reading PSUM
            hiv = h_hi[co][:, nt * 8 + 1:nt * 8 + 9, 1:65]
            nc.scalar.activation(hiv, ps[:], ACT.Relu,
                                 bias=b1_sb[:, co:co + 1], scale=1.0)
            hstg = ptrans.tile([128, 512], F32, name="hstg", tag="hstg", bufs=2)
            nc.scalar.activation(hstg[:], ps[:], ACT.Relu,
                                 bias=b1_sb[:, co:co + 1], scale=1.0)
            nc.vector.tensor_sub(h_lo[co][:, nt * 8 + 1:nt * 8 + 9, 1:65],
                                 hstg[:], hiv)

    def conv2_strip(m):
        # psA rows = [w2h*h_hi (2) ; w2l*h_hi (2)], psB = w2h*h_lo. The
        # pair-sum across partitions happens post-transpose, in the free dim.
        # both accumulators packed into one PSUM bank: psA at partitions 0-3
        # (col group 0), psB at 32-33 (col group 1) -> they run concurrently
        ps6 = psmall.tile([34, 512], F32, name="c2ps", tag="c2ps")
        psA = ps6[0:4, :]
        psB = ps6[32:34, :]
        for t9 in range(9):
            dy, dx = t9 // 3 - 1, t9 % 3 - 1
            for ci in range(2):
                c = t9 * 2 + ci
                rhs_hi = h_hi[ci][:, m * 8 + 1 + dy:m * 8 + 9 + dy, 1 + dx:65 + dx]
                rhs_lo = h_lo[ci][:, m * 8 + 1 + dy:m * 8 + 9 + dy, 1 + dx:65 + dx]
                first = (c == 0)
                last = (c == 17)
                nc.tensor.matmul(psA, w2p_sb[:, c * 4:c * 4 + 4],
                                 rhs_hi, start=first, stop=last)
                nc.tensor.matmul(psB, w2p_sb[:, c * 4:c * 4 + 2],
                                 rhs_lo, start=first, stop=last,
                                 tile_position=(0, 32))
        offA = pgather.tile([4, 512], F32, name="offA", tag="offA", bufs=2)
        offB = pgather.tile([2, 512], F32, name="offB", tag="offB", bufs=2)
        nc.vector.tensor_copy(offA[:], psA)
        nc.vector.tensor_copy(offB[:], psB)
        return offA, offB

    def grid_strip(m, offAB):
        offA, offB = offAB
        # pixel-partition layout: pixel p = j*128 + i -> [i, j], j in [0,32)
        pst6 = ptpsum.tile([128, 4, 6], F32, name="offT_psum", tag="offT",
                           bufs=2)
        for jj in range(4):
            nc.tensor.transpose(pst6[:, jj, 0:4],
                                offA[:, jj * 128:(jj + 1) * 128], id_sb[:4, :4])
            nc.tensor.transpose(pst6[:, jj, 4:6],
                                offB[:, jj * 128:(jj + 1) * 128], id_sb[:2, :2])
        J = slice(4 * m, 4 * m + 4)        # j-cols of [128,32] tiles
        X = slice(8 * m, 8 * m + 8)        # cols of ixiy [128,64]
        i3 = ixiy[:, X].rearrange("p (j c) -> p j c", c=2)
        # ix/iy = 32*offset + base (scale folded into w2 on host; bxy = base)
        nc.vector.tensor_copy(i3, pst6[:, :, 0:2])
        nc.vector.tensor_add(i3, i3, pst6[:, :, 2:4])
        nc.vector.tensor_add(i3, i3, pst6[:, :, 4:6])
        nc.vector.tensor_add(ixiy[:, X], ixiy[:, X], bxy_sb[:, X])
        ix = ixiy[:, 8 * m:8 * m + 7:2]
        iy = ixiy[:, 8 * m + 1:8 * m + 8:2]

        # exact floor via int cast + correction
        nc.vector.tensor_copy(xi_i[:, J], ix)
        nc.vector.tensor_copy(fx0[:, J], xi_i[:, J])
        nc.vector.tensor_tensor(corr[:, J], fx0[:, J], ix, op=ALU.is_gt)
        nc.vector.tensor_sub(fx0[:, J], fx0[:, J], corr[:, J])
        nc.vector.tensor_copy(yi_i[:, J], iy)
        nc.vector.tensor_copy(fy0[:, J], yi_i[:, J])
        nc.vector.tensor_tensor(corr2[:, J], fy0[:, J], iy, op=ALU.is_gt)
        nc.vector.tensor_sub(fy0[:, J], fy0[:, J], corr2[:, J])

        nc.vector.tensor_sub(wx[:, J], ix, fx0[:, J])
        nc.vector.tensor_sub(wy[:, J], iy, fy0[:, J])

        def valid01(src, v0, v1):
            nc.vector.tensor_scalar(va[:, J], src[:, J], 0.0, None, op0=ALU.is_ge)
            nc.vector.tensor_scalar(vb[:, J], src[:, J], 63.0, None, op0=ALU.is_le)
            nc.vector.tensor_mul(v0[:, J], va[:, J], vb[:, J])
            nc.vector.tensor_scalar(va[:, J], src[:, J], -1.0, None, op0=ALU.is_ge)
            nc.vector.tensor_scalar(vb[:, J], src[:, J], 62.0, None, op0=ALU.is_le)
            nc.vector.tensor_mul(v1[:, J], va[:, J], vb[:, J])

        valid01(fx0, vx0, vx1)
        valid01(fy0, vy0, vy1)

        # clamped addresses (+1 guard-row shift folded into xc1)
        nc.vector.tensor_scalar(xc1[:, J], fx0[:, J], -1.0, 64.0,
                                op0=ALU.max, op1=ALU.min)
        nc.vector.tensor_scalar_add(xc1[:, J], xc1[:, J], 1.0)
        nc.vector.tensor_scalar(yc0[:, J], fy0[:, J], 0.0, 63.0,
                                op0=ALU.max, op1=ALU.min)
        nc.vector.tensor_scalar(yc1[:, J], fy0[:, J], 1.0, 0.0,
                                op0=ALU.add, op1=ALU.max)
        nc.vector.tensor_scalar_min(yc1[:, J], yc1[:, J], 63.0)

        nc.vector.scalar_tensor_tensor(idxA_f[:, J], yc0[:, J], 64.0, xc1[:, J],
                                       op0=ALU.mult, op1=ALU.add)
        nc.vector.scalar_tensor_tensor(idxB_f[:, J], yc1[:, J], 64.0, xc1[:, J],
                                       op0=ALU.mult, op1=ALU.add)
        nc.vector.tensor_copy(idxA[:, J], idxA_f[:, J])
        nc.vector.tensor_copy(idxB[:, J], idxB_f[:, J])

        # bilinear weights, validity folded in
        nc.vector.tensor_scalar(u[:, J], wx[:, J], -1.0, 1.0,
                                op0=ALU.mult, op1=ALU.add)
        nc.vector.tensor_mul(u[:, J], u[:, J], vx0[:, J])
        nc.vector.tensor_scalar(v[:, J], wy[:, J], -1.0, 1.0,
                                op0=ALU.mult, op1=ALU.add)
        nc.vector.tensor_mul(v[:, J], v[:, J], vy0[:, J])
        nc.vector.tensor_mul(wxv[:, J], wx[:, J], vx1[:, J])
        nc.vector.tensor_mul(wyv[:, J], wy[:, J], vy1[:, J])
        nc.vector.tensor_mul(w00[:, J], u[:, J], v[:, J])
        nc.vector.tensor_mul(w01[:, J], wxv[:, J], v[:, J])
        nc.vector.tensor_mul(w10[:, J], u[:, J], wyv[:, J])
        nc.vector.tensor_mul(w11[:, J], wxv[:, J], wyv[:, J])

        # issue the row gathers for this strip's 4 j-chunks now; the
        # bilinear combine happens one stage later
        gs = []
        for jj in range(4):
            j = 4 * m + jj
            gA = pgather.tile([128, 512], BF16, name="gA", tag="gA", bufs=8)
            gB = pgather.tile([128, 512], BF16, name="gB", tag="gB", bufs=8)
            nc.gpsimd.indirect_dma_start(
                out=gA[:], out_offset=None, in_=prm["xT2"][:],
                in_offset=bass.IndirectOffsetOnAxis(ap=idxA[:, j:j + 1], axis=0))
            nc.gpsimd.indirect_dma_start(
                out=gB[:], out_offset=None, in_=prm["xT2"][:],
                in_offset=bass.IndirectOffsetOnAxis(ap=idxB[:, j:j + 1], axis=0))
            gs.append((gA, gB))
        return gs

    def diff_strip(m, gs):
        xlw = []
        for co in range(2):
            t = pgather.tile([128, 8, 64], F32, name=f"xlw{co}",
                             tag=f"xlw{co}", bufs=2)
            nc.sync.dma_start(t[:], prm["xl"][co * 128:(co + 1) * 128,
                                              m * 8:m * 8 + 8, :])
            xlw.append(t)
        for jj in range(4):
            j = 4 * m + jj
            gA, gB = gs[jj]
            acc = pgather.tile([128, 256], F32, name="acc", tag="acc", bufs=2)
            nc.vector.tensor_scalar_mul(acc[:], gA[:, 0:256], w00[:, j:j + 1])
            nc.vector.scalar_tensor_tensor(acc[:], gA[:, 256:512],
                                           w01[:, j:j + 1], acc[:],
                                           op0=ALU.mult, op1=ALU.add)
            nc.vector.scalar_tensor_tensor(acc[:], gB[:, 0:256],
                                           w10[:, j:j + 1], acc[:],
                                           op0=ALU.mult, op1=ALU.add)
            nc.vector.scalar_tensor_tensor(acc[:], gB[:, 256:512],
                                           w11[:, j:j + 1], acc[:],
                                           op0=ALU.mult, op1=ALU.add)
            # transpose [128px, 256ch] to channel-major, diff = x_low - aligned
            for co in range(2):
                pt = ptpsum.tile([128, 128], F32, name="alT_psum", tag="alT")
                nc.tensor.transpose(pt[:], acc[:, co * 128:(co + 1) * 128],
                                    id_sb[:])
                nc.vector.tensor_sub(
                    diff_pad[co][:, 2 * j + 1:2 * j + 3, 1:65],
                    xlw[co][:, 2 * jj:2 * jj + 2, :], pt[:])

    # ---------------- dw-branch emitters ----------------
    def dw_alloc(k):
        dwk_sb = pback.tile([128, 4608], BF16, name="dwk_sb", tag="dwk",
                            bufs=2)
        nc.scalar.dma_start(dwk_sb[:], prm[f"dwT{k}"][:])
        xd = []
        pooled_parts = []
        for co in range(2):
            xd_t = pback.tile([128, HW], BF16, name=f"xd{co}", tag=f"xd{co}",
                              bufs=2)
            xd.append(xd_t)
            pp_t = pc.tile([128, 8], F32, name=f"pooled_parts{co}",
                           tag=f"pooled_parts{co}", bufs=2)
            pooled_parts.append(pp_t)
        return k, dwk_sb, xd, pooled_parts

    def dw_group(kctx, co, nt):
        k, dwk_sb, xd, pooled_parts = kctx
        ps = ppsum.tile([128, 512], F32, name="dwpsum", tag="c1psum")
        first = True
        for t9 in range(9):
            dy, dx = t9 // 3 - 1, t9 % 3 - 1
            for ci in range(2):
                col = ((t9 * 2 + ci) * 2 + co) * 128
                nc.tensor.matmul(
                    ps[:],
                    dwk_sb[:, col:col + 128],
                    diff_pad[ci][:, nt * 8 + 1 + dy:nt * 8 + 9 + dy,
                                 1 + dx:65 + dx],
                    start=first, stop=(t9 == 8 and ci == 1))
                first = False
        nc.scalar.activation(
            xd[co][:, nt * 512:(nt + 1) * 512], ps[:],
            ACT.Identity, bias=db_sb[:, 2 * k + co:2 * k + co + 1],
            scale=1.0, accum_out=pooled_parts[co][:, nt:nt + 1])

    # ---------------- front-end pipeline ----------------
    conv2_out = {}
    grid_out = {}
    for nt in range(8):
        conv1_strip(nt)
        if nt >= 1:
            conv2_out[nt - 1] = conv2_strip(nt - 1)
        if nt >= 2:
            grid_out[nt - 2] = grid_strip(nt - 2, conv2_out.pop(nt - 2))
        if nt >= 3:
            diff_strip(nt - 3, grid_out.pop(nt - 3))
    conv2_out[7] = conv2_strip(7)  # last user of h / ptrans

    ptrans.release()
    pback = tc.alloc_tile_pool(name="pback", bufs=1)
    ctx_pools.append(pback)
    fused_pad = []
    for co in range(2):
        t = pback.tile([128, PH, PW], BF16, name=f"fused_pad{co}")
        memset_border(t)
        fused_pad.append(t)

    # pipeline drain interleaved with the first dw-conv groups so the PE
    # FIFO never stalls on the gather chain's transposes
    k0 = dw_alloc(0)
    grid_out[6] = grid_strip(6, conv2_out.pop(6))
    dw_group(k0, 0, 0)
    dw_group(k0, 0, 1)
    diff_strip(5, grid_out.pop(5))
    dw_group(k0, 0, 2)
    grid_out[7] = grid_strip(7, conv2_out.pop(7))
    dw_group(k0, 0, 3)
    diff_strip(6, grid_out.pop(6))
    dw_group(k0, 1, 0)
    dw_group(k0, 1, 1)
    diff_strip(7, grid_out.pop(7))
    for nt in range(4, 8):
        dw_group(k0, 0, nt)
    for nt in range(2, 8):
        dw_group(k0, 1, nt)

    # Software-pipelined: emit branch k+1's matmuls before branch k's SE
    # chain so the tiny SE matvecs (which wait on pooled stats) never stall
    # the PE queue between the big conv groups. xd / pooled tags use bufs=2
    # so branch k's data survives while branch k+1 computes.
    def emit_dw_matmuls(k):
        kctx = dw_alloc(k)
        for co in range(2):
            for nt in range(8):
                dw_group(kctx, co, nt)
        return kctx[2], kctx[3]

    def emit_se(k, xd, pooled_parts):
        # SE block (tiny matvecs); mean 1/HW folded into se1T on host.
        # The 8->1 reduction rides the ACT accumulator (the DVE queue is
        # busy with fused updates right when this needs to run).
        pooled = []
        for co in range(2):
            p_t = pc.tile([128, 1], F32, name=f"pooled{co}", tag=f"pooled{co}")
            pdmp = pc.tile([128, 8], F32, name=f"pdump{co}", tag=f"pdump{co}",
                           bufs=2)
            nc.scalar.activation(pdmp[:], pooled_parts[co][:], ACT.Identity,
                                 scale=1.0, accum_out=p_t[:])
            pooled.append(p_t)
        pse = psmall.tile([128, 2], F32, name="pse", tag="pse")
        nc.tensor.matmul(pse[0:64, 0:1], se1_sb[:, k * 128:k * 128 + 64],
                         pooled[0][:], start=True, stop=False)
        nc.tensor.matmul(pse[0:64, 0:1], se1_sb[:, k * 128 + 64:k * 128 + 128],
                         pooled[1][:], start=False, stop=True)
        h1 = pc.tile([64, 1], F32, name="h1", tag="h1")
        nc.scalar.activation(h1[:], pse[0:64, 0:1], ACT.Relu,
                             bias=se1b_sb[:, k:k + 1], scale=1.0)
        for co in range(2):
            nc.tensor.matmul(pse[:, 1:2],
                             se2_sb[:, (k * 2 + co) * 128:(k * 2 + co + 1) * 128],
                             h1[:], start=True, stop=True)
            s_t = pc.tile([128, 1], F32, name=f"s{co}", tag=f"s{co}")
            nc.scalar.activation(s_t[:], pse[:, 1:2], ACT.Sigmoid,
                                 bias=se2b_sb[:, 2 * k + co:2 * k + co + 1],
                                 scale=1.0)
            # fused += xd * s (split in half-image chunks so the attention
            # conv's first strips unblock after the first chunk)
            xd3 = xd[co][:].rearrange("p (h w) -> p h w", h=H)
            for r0, r1 in [(0, 32), (32, 64)]:
                fslice = fused_pad[co][:, 1 + r0:1 + r1, 1:1 + W]
                if k == 0:
                    nc.vector.tensor_scalar_mul(
                        fslice, xd3[:, r0:r1, :], s_t[:])
                else:
                    nc.vector.scalar_tensor_tensor(
                        fslice, xd3[:, r0:r1, :], s_t[:], fslice,
                        op0=ALU.mult, op1=ALU.add)

    prev = (k0[2], k0[3])
    for k in range(1, 3):
        cur = emit_dw_matmuls(k)
        emit_se(k - 1, *prev)
        prev = cur
    emit_se(2, *prev)

    # ---------------- attention + final ----------------
    for nt in range(8):
        attn = pback.tile([128, 512], BF16, name="attn", tag="attn", bufs=2)
        ps = ppsum.tile([128, 512], F32, name="sapsum", tag="c1psum")
        first = True
        for t9 in range(9):
            dy, dx = t9 // 3 - 1, t9 % 3 - 1
            for ci in range(2):
                col = (t9 * 2 + ci) * 128
                nc.tensor.matmul(
                    ps[:],
                    saw_sb[:, col:col + 128],
                    fused_pad[ci][:, nt * 8 + 1 + dy:nt * 8 + 9 + dy,
                                  1 + dx:65 + dx],
                    start=first, stop=(t9 == 8 and ci == 1))
                first = False
        nc.scalar.activation(attn[:], ps[:], ACT.Sigmoid, bias=sab_sb[:, 0:1],
                             scale=1.0)
        for co in range(2):
            xlt = pback.tile([128, 512], F32, name="xlt", tag="xlt", bufs=2)
            nc.sync.dma_start(
                xlt[:], prm["xl"][co * 128:(co + 1) * 128, nt * 8:(nt + 1) * 8, :])
            ot = pback.tile([128, 512], F32, name="ot", tag="ot", bufs=2)
            nc.vector.tensor_mul(
                ot[:], attn[:],
                diff_pad[co][:, nt * 8 + 1:nt * 8 + 9, 1:65])
            nc.vector.tensor_add(ot[:], ot[:], xlt[:])
            nc.sync.dma_start(
                prm["out"][co * 128:(co + 1) * 128, nt * 512:(nt + 1) * 512],
                ot[:])

    for p in reversed(ctx_pools):
        p.release()


def _build(repeat):
    nc = bacc.Bacc()
    prm = {}

    def din(name, shape, dt=F32):
        prm[name] = nc.declare_dram_parameter(name, list(shape), dt,
                                              isOutput=False)

    din("xl", [C, H, W])
    for nm in ["xlhi", "xllo", "xhhi", "xhlo"]:
        din(nm, [C, H, W], BF16)
    din("xT2", [TBL_ROWS, 512], BF16)
    din("w1Th", [128, 9216], BF16)
    din("w1Tl", [128, 9216], BF16)
    din("b1", [128, 2])
    din("w2p", [128, 72], BF16)
    din("bxy", [128, 64])
    din("ident", [128, 128])
    for k in range(3):
        din(f"dwT{k}", [128, 4608], BF16)
    din("db2", [128, 6])
    din("se1T", [128, 384])
    din("se1b2", [64, 3])
    din("se2T", [64, 768])
    din("se2b2", [128, 6])
    din("sawT", [128, 2304], BF16)
    din("sab_bc", [128, 1])
    prm["out"] = nc.declare_dram_parameter("out", [C, HW], F32, isOutput=True)

    with tile.TileContext(nc) as tc:
        pc_const, cn = _load_consts(nc, tc, prm)
        _emit_body(nc, tc, prm, cn, first_iter=True)
        if repeat > 1:
            with tc.For_i(0, repeat - 1, 1):
                _emit_body(nc, tc, prm, cn, first_iter=False)
        pc_const.release()
    nc.finalize()
    return nc


def _prep_inputs(x_low, x_high, a1w, a1b, bn_g, bn_b, bn_m, bn_v, a2w, a2b,
                 dw, db, se1w, se1b, se2w, se2b, saw, sab):
    """Host-side weight prep shared by all cores + per-core activation prep."""
    import ml_dtypes
    f32 = np.float32
    bf16 = ml_dtypes.bfloat16
    # conv1 with BN folded
    scale = (bn_g / np.sqrt(bn_v + EPS)).astype(f32)  # [256]
    w1f = (a1w * scale[:, None, None, None]).astype(f32)  # [256,512,3,3]
    b1f = ((a1b - bn_m) * scale + bn_b).astype(f32)  # [256]
    # host lhsT layout [k(128), ty,tx, ci(4), co(2), m(128)] -> [128, 9216]
    arr = w1f.reshape(2, 128, 4, 128, 3, 3)  # [co, m, ci, k, ty, tx]
    w1T = np.ascontiguousarray(arr.transpose(3, 0, 4, 5, 2, 1)).reshape(128, 9216)
    w1Th = w1T.astype(bf16)
    w1Tl = (w1T - w1Th.astype(np.float32)).astype(bf16)
    b1h = np.ascontiguousarray(b1f.reshape(2, 128).T)  # [128, 2]

    # conv2, grid scale W/2 = 32 folded in; hi/lo M-packed [w2h|w2l] per chunk
    w2f = (a2w * 32.0).astype(f32)  # [2, 256, 3, 3]
    arr = w2f.reshape(2, 2, 128, 3, 3)  # [m, ci, k, ty, tx]
    w2T = np.ascontiguousarray(arr.transpose(2, 3, 4, 1, 0))  # [k,ty,tx,ci,m]
    w2h = w2T.astype(bf16)
    w2l = (w2T - w2h.astype(np.float32)).astype(bf16)
    w2p = np.concatenate([w2h.reshape(128, 18, 2), w2l.reshape(128, 18, 2)],
                         axis=2)  # [k, chunk, 4]
    w2p = np.ascontiguousarray(w2p).reshape(128, 72)

    # base grid (+a2b*32): pixel p = j*128+i ; h=p//64, w=p%64
    lin = np.linspace(-1.0, 1.0, 64, dtype=f32)
    pidx = (np.arange(32)[None, :] * 128 + np.arange(128)[:, None])  # [128,32]
    bx = ((lin[pidx // 64] + 1.0) * 32.0 - 0.5 + 32.0 * f32(a2b[0])).astype(f32)
    by = ((lin[pidx % 64] + 1.0) * 32.0 - 0.5 + 32.0 * f32(a2b[1])).astype(f32)
    bxy = np.empty((128, 64), f32)
    bxy[:, 0::2] = bx
    bxy[:, 1::2] = by

    # diff convs (bf16)
    dwT = []
    for k in range(3):
        arr = dw[k].astype(f32).reshape(2, 128, 2, 128, 3, 3)  # [co,m,ci,kk,ty,tx]
        dwT.append(np.ascontiguousarray(
            arr.transpose(3, 4, 5, 2, 0, 1)).reshape(128, 4608).astype(bf16))
    db2 = np.ascontiguousarray(db.astype(f32).reshape(3, 2, 128).transpose(2, 0, 1)
                               ).reshape(128, 6)

    # SE (mean 1/HW folded into se1T)
    se1T = np.ascontiguousarray(
        (se1w.astype(f32) / HW).transpose(0, 2, 1).reshape(3, 2, 128, 64)
        .transpose(2, 0, 1, 3)).reshape(128, 384)
    se1b2 = np.ascontiguousarray(se1b.astype(f32).T)  # [64, 3]
    se2T = np.ascontiguousarray(
        se2w.astype(f32).transpose(0, 2, 1).reshape(3, 64, 2, 128)
        .transpose(1, 0, 2, 3)).reshape(64, 768)
    se2b2 = np.ascontiguousarray(se2b.astype(f32).reshape(3, 2, 128)
                                 .transpose(2, 0, 1)).reshape(128, 6)

    # attention conv, weights replicated to M=128 (bf16)
    arr = saw.astype(f32).reshape(1, 2, 128, 3, 3)  # [m=1, ci, k, ty, tx]
    arr = np.broadcast_to(arr, (128, 2, 128, 3, 3))  # replicate m
    sawT = np.ascontiguousarray(
        arr.transpose(2, 3, 4, 1, 0)).reshape(128, 2304).astype(bf16)
    sab_bc = np.full((128, 1), f32(sab[0]), f32)

    shared = dict(w1Th=w1Th, w1Tl=w1Tl, b1=b1h, w2p=w2p, bxy=bxy,
                  ident=np.eye(128, dtype=f32),
                  dwT0=dwT[0], dwT1=dwT[1], dwT2=dwT[2], db2=db2,
                  se1T=se1T, se1b2=se1b2, se2T=se2T, se2b2=se2b2,
                  sawT=sawT, sab_bc=sab_bc)

    in_maps = []
    for b in range(B):
        xlb = np.ascontiguousarray(x_low[b].astype(f32))
        xhb = x_high[b].astype(f32)
        xlhi = xlb.astype(bf16)
        xllo = (xlb - xlhi.astype(f32)).astype(bf16)
        xhhi = xhb.astype(bf16)
        xhlo = (xhb - xhhi.astype(f32)).astype(bf16)
        XT = np.ascontiguousarray(xhb.reshape(C, HW).T)  # [4096, 256]
        XT2 = np.zeros((TBL_ROWS, 512), f32)
        XT2[1:1 + HW, :256] = XT
        XT2[0:HW, 256:] = XT
        m = dict(shared)
        m["xl"] = xlb
        m["xlhi"] = np.ascontiguousarray(xlhi)
        m["xllo"] = np.ascontiguousarray(xllo)
        m["xhhi"] = np.ascontiguousarray(xhhi)
        m["xhlo"] = np.ascontiguousarray(xhlo)
        m["xT2"] = XT2.astype(bf16)
        in_maps.append(m)
    return in_maps


_last_results = None


def kernel(**inputs):
    global _last_results
    repeat = int(os.environ.get("KERNEL_REPEAT", "1"))
    if repeat not in _nc_cache:
        _nc_cache[repeat] = _build(repeat)
    nc = _nc_cache[repeat]
    in_maps = _prep_inputs(**inputs)
    res = run_bass_kernel_spmd(nc, in_maps, list(range(NCORES)))
    _last_results = res
    out = np.stack([res.results[b]["out"].reshape(C, H, W) for b in range(B)])
    return out.astype(np.float32)


if __name__ == "__main__":
    import reference
    inputs = {k: np.asarray(v) for k, v in reference.setup_inputs().items()}
    expected = np.asarray(reference.reference(**inputs))
    actual = kernel(**inputs)
    err = np.abs(actual - expected).max()
    rel = err / np.abs(expected).max()
    print(f"abs err: {err:.4e}  rel err: {rel:.4e}")
